# revision 3
# baseline (speedup 1.0000x reference)
"""Chamfer distance loss kernel v2 for Trainium2 (8 NeuronCores, Bass/Tile).

Problem: A, B [4, 8192, 3] f32 point clouds ->
    mean_b( mean_n min_m ||A[b,n]-B[b,m]|| + mean_m min_n ||.|| ) / 12.8

Strategy (per core = one batch x one half of A's rows):
  - [4096 x 8192] squared-distance block via K=13 float32r matmuls
    (11-bit hi/lo split => fp32-exact d^2 at 1 PE cycle/row).
  - All consumption uses the negated convention (-d^2, MAX = min of d^2).
  - Per row tile (128 rows x 8192 cols = 4 PSUM groups of 2048): ACT stages
    each group to f16 (scale=-1) — ACT is the only cheap PSUM reader
    (0.83 ns/elem); everything else must run on DVE since Pool/gpsimd
    cannot execute tensor ops and DMA cannot accumulate min/max or read
    PSUM on this toolchain.
  - DVE consumes the staged f16 tile at 2x (0.52 ns/elem) with max-width
    ops: one 8192-wide tensor_tensor MAX into persistent bmin [128, 8192]
    (B-side col-min), then a 3-op fold chain 8192->4096->2048->1024 into
    amin_w [128, 32, 1024] (A-side row-min partials).
  - No on-device cross-partition reduction: bmin ships raw, amin_w ships
    progressively in 8 chunks; the host does the cheap tails: fold
    1024-wide slots, min across 128 partitions, core/batch combine,
    clamp/sqrt/means. min/sqrt/mean commute with the sharding.
  - Engine busy/row-tile: DVE 8.6us (bottleneck), ACT 7.9us, PE 3.4-6.8us.
"""
import os
import hashlib
import shutil
import numpy as np
from contextlib import ExitStack

import concourse.bass as bass
import concourse.tile as tile
import concourse.mybir as mybir
import concourse.bass2jax as bass2jax
from concourse import bass_utils
from concourse.vector_clock import ScopedClock

# ---------------------------------------------------------------------------
# Patch 1: walrus encodes at most ONE sync wait per TPB instruction
# ("Too many sync wait commands"). Tile attaches several (incl. the tail
# drain). Split extras onto preceding same-engine EventSemaphore/Drain
# instructions.
# ---------------------------------------------------------------------------


def _patched_drain_and_barrier(self, tick_clock, wait_clock):
    nc = self.nc
    drain_inst = nc.sync.drain()
    wait_clock.add_sem_waits(
        drain_inst.ins, ScopedClock({None: tick_clock.global_clock})
    )
    si = drain_inst.ins.sync_info
    if si is not None and len(si.on_wait) > 1:
        waits = list(si.on_wait)
        drain_inst.ins.sync_info = mybir.SyncInfo(
            on_wait=waits[:1], on_update=list(si.on_update)
        )
        for i in range(1, len(waits)):
            extra = nc.sync.drain()
            extra.ins.sync_info = mybir.SyncInfo(
                on_wait=waits[i:i + 1], on_update=[]
            )

    nc.all_engine_barrier()
    assert self.sems is not None
    popped = nc._tile_sem_poison_stack.pop()
    assert popped is self._sem_poison
    nc.clear_and_free_semaphores(list(self.sems.allocated().values()))
    nc.all_engine_barrier()


tile.TileContext._drain_and_barrier = _patched_drain_and_barrier

_split_counter = [0]


def _split_multi_waits(nc):
    for f in nc.m.functions:
        for bb in f.blocks:
            insts = bb.instructions
            out = []
            changed = False
            for inst in insts:
                si = inst.sync_info
                if si is not None and len(si.on_wait) > 1:
                    waits = list(si.on_wait)
                    for w in waits[:-1]:
                        _split_counter[0] += 1
                        ev = mybir.InstEventSemaphore(
                            name=f"evsplit_{_split_counter[0]}"
                        )
                        ev.engine = inst.engine
                        ev.sync_info = mybir.SyncInfo(on_wait=[w], on_update=[])
                        out.append(ev)
                    inst.sync_info = mybir.SyncInfo(
                        on_wait=waits[-1:], on_update=list(si.on_update)
                    )
                    changed = True
                out.append(inst)
            if changed:
                bb.instructions = out


# ---------------------------------------------------------------------------
# Patch 2: disk-cache compiled NEFFs by BIR hash so repeated kernel() calls
# and processes skip the multi-minute walrus compile.
# ---------------------------------------------------------------------------

_NEFF_CACHE_DIR = os.environ.get("BASS_NEFF_CACHE_DIR", "/tmp/bass_neff_cache")
_orig_compile_bir_kernel = bass_utils.compile_bir_kernel


def _cached_compile_bir_kernel(bir_json, tmpdir, neff_name="file.neff"):
    try:
        os.makedirs(_NEFF_CACHE_DIR, exist_ok=True)
        key = hashlib.sha256(bir_json).hexdigest()
        cpath = os.path.join(_NEFF_CACHE_DIR, f"{key}_{neff_name}")
        dst_dir = os.path.join(tmpdir, "sg00")
        dst = os.path.join(dst_dir, neff_name)
        if os.path.exists(cpath):
            os.makedirs(dst_dir, exist_ok=True)
            shutil.copyfile(cpath, dst)
            return dst
        out = _orig_compile_bir_kernel(bir_json, tmpdir, neff_name)
        try:
            shutil.copyfile(out, cpath)
        except OSError:
            pass
        return out
    except Exception:
        return _orig_compile_bir_kernel(bir_json, tmpdir, neff_name)


bass_utils.compile_bir_kernel = _cached_compile_bir_kernel
bass2jax.compile_bir_kernel = _cached_compile_bir_kernel

# ---------------------------------------------------------------------------
# Kernel build
# ---------------------------------------------------------------------------

F16 = mybir.dt.float16
F32 = mybir.dt.float32
F32R = mybir.dt.float32r
MIN = mybir.AluOpType.min
MAX = mybir.AluOpType.max
COPYF = mybir.ActivationFunctionType.Copy

KK = 13        # hi/lo-split augmented contraction dim
P = 128
CHUNK = 512    # PSUM bank free size (fp32)
GW = 2048      # group width (4 banks)
NG = 4         # groups per row tile
BATCH = 4
N = 8192
HALF = N // 2
RT = HALF // P  # 32 row tiles
N_CORES = 8
SPLIT_BITS = 11
NEG_INIT = -60000.0  # f16-representable, below any -d^2

def _build_nc():
    nc = bass.Bass(trn_type="TRN2")
    lhsT_d = nc.dram_tensor("lhsT", [KK, HALF], F32, kind="ExternalInput")
    rhsB_d = nc.dram_tensor("rhsB", [KK, N], F32, kind="ExternalInput")
    aw_d = nc.dram_tensor("aw", [P, RT, 1024], F16, kind="ExternalOutput")
    bmin_d = nc.dram_tensor("bmin", [P, N], F16, kind="ExternalOutput")

    with tile.TileContext(nc) as tc:
        with ExitStack() as ctx:
            consts = ctx.enter_context(tc.tile_pool(name="consts", bufs=1))
            psum = ctx.enter_context(
                tc.tile_pool(name="psum", bufs=2, space="PSUM")
            )
            tpool = ctx.enter_context(tc.tile_pool(name="tpool", bufs=2))
            scr = ctx.enter_context(tc.tile_pool(name="scr", bufs=2))

            lhs_sb = consts.tile([KK, HALF], F32R)
            nc.gpsimd.dma_start(out=lhs_sb, in_=lhsT_d[:, :])
            rhs_sb = consts.tile([KK, N], F32R)
            for g0 in range(4):
                nc.gpsimd.dma_start(
                    out=rhs_sb[:, g0 * 2048:(g0 + 1) * 2048],
                    in_=rhsB_d[:, g0 * 2048:(g0 + 1) * 2048],
                )

            bmin = consts.tile([P, N], F16)
            # init bmin below any -d^2 value; per-group owners then MAX in.
            nc.vector.memset(bmin, NEG_INIT)
            amin_w = consts.tile([P, RT, 1024], F16)

            for i in range(RT):
                T = tpool.tile([P, N], F16, tag="T")
                for g in range(NG):
                    pt = psum.tile([P, GW], F32, tag="pt")
                    for q in range(NG):
                        j = g * NG + q
                        nc.tensor.matmul(
                            pt[:, q * CHUNK:(q + 1) * CHUNK],
                            lhs_sb[:, i * P:(i + 1) * P],
                            rhs_sb[:, j * CHUNK:(j + 1) * CHUNK],
                            start=True,
                            stop=True,
                        )
                    nc.scalar.activation(
                        out=T[:, g * GW:(g + 1) * GW], in_=pt,
                        func=COPYF, scale=-1.0,
                    )
                # B-side: one full-width accumulate (col-min as MAX of -d^2)
                nc.vector.tensor_tensor(out=bmin, in0=T, in1=bmin, op=MAX)
                # A-side fold chain to 1024-wide slots
                c1 = scr.tile([P, N // 2], F16, tag="c1")
                nc.vector.tensor_tensor(
                    out=c1, in0=T[:, 0:N // 2], in1=T[:, N // 2:N], op=MAX
                )
                c2 = scr.tile([P, GW], F16, tag="c2")
                nc.vector.tensor_tensor(
                    out=c2, in0=c1[:, 0:GW], in1=c1[:, GW:2 * GW], op=MAX
                )
                nc.vector.tensor_tensor(
                    out=amin_w[:, i, :], in0=c2[:, 0:1024], in1=c2[:, 1024:2048],
                    op=MAX,
                )
                # ship amin_w progressively in 8 chunks of 4 tiles
                if i % 4 == 3:
                    c = i // 4
                    nc.sync.dma_start(
                        out=aw_d[:, c * 4:(c + 1) * 4, :],
                        in_=amin_w[:, c * 4:(c + 1) * 4, :],
                    )
            nc.sync.dma_start(out=bmin_d[:, :], in_=bmin)
    _split_multi_waits(nc)
    return nc


_NC = None


def _get_nc():
    global _NC
    if _NC is None:
        _NC = _build_nc()
    return _NC


def _round_mant(v, bits=SPLIT_BITS):
    m, e = np.frexp(v.astype(np.float64))
    return np.ldexp(np.round(m * (1 << bits)) / (1 << bits), e).astype(np.float32)


def _host_prep_core(Asub, Bfull):
    """Build the K=13 hi/lo-split augmented operands (all 11-bit exact)."""
    a2 = (Asub.astype(np.float32) ** 2).sum(axis=1)
    b2 = (Bfull.astype(np.float32) ** 2).sum(axis=1)
    ah = _round_mant(Asub.T)
    al = (Asub.T - ah).astype(np.float32)
    bh = _round_mant(Bfull.T)
    bl = (Bfull.T - bh).astype(np.float32)
    a2h = _round_mant(a2)
    a2l = (a2 - a2h).astype(np.float32)
    b2h = _round_mant(b2)
    b2l = (b2 - b2h).astype(np.float32)

    lhsT = np.empty((KK, Asub.shape[0]), np.float32)
    rhsB = np.empty((KK, Bfull.shape[0]), np.float32)
    lhsT[0:3] = ah
    rhsB[0:3] = -2.0 * bh
    lhsT[3:6] = ah
    rhsB[3:6] = -2.0 * bl
    lhsT[6:9] = al
    rhsB[6:9] = -2.0 * bh
    lhsT[9] = a2h
    rhsB[9] = 1.0
    lhsT[10] = a2l
    rhsB[10] = 1.0
    lhsT[11] = 1.0
    rhsB[11] = b2h
    lhsT[12] = 1.0
    rhsB[12] = b2l
    return {"lhsT": lhsT, "rhsB": rhsB}


def kernel(A, B):
    A = np.ascontiguousarray(np.asarray(A, dtype=np.float32))
    B = np.ascontiguousarray(np.asarray(B, dtype=np.float32))
    nc = _get_nc()

    in_maps = []
    for c in range(N_CORES):
        b, h = divmod(c, 2)
        in_maps.append(_host_prep_core(A[b, h * HALF:(h + 1) * HALF], B[b]))

    res = bass_utils.run_bass_kernel_spmd(
        nc, in_maps, core_ids=list(range(N_CORES))
    )

    cham = []
    for b in range(BATCH):
        a_rows = []   # min d^2 per A row
        b_sq = None   # columnwise min d^2 over all rows
        for h in range(2):
            r = res.results[2 * b + h]
            aw = np.asarray(r["aw"], dtype=np.float32)      # [128, 32, 1024] (-d^2)
            bm = np.asarray(r["bmin"], dtype=np.float32)    # [128, 8192] (-d^2)
            a_d2 = -aw.max(axis=2)                          # [128, 32]
            # row index = i*128 + p  ->  [32, 128] -> flat
            a_rows.append(a_d2.T.reshape(-1))
            cb = -bm.max(axis=0)                            # [8192]
            b_sq = cb if b_sq is None else np.minimum(b_sq, cb)
        a_sq = np.concatenate(a_rows)
        da = np.sqrt(np.maximum(a_sq, 0.0))
        db = np.sqrt(np.maximum(b_sq, 0.0))
        cham.append(da.mean() + db.mean())

    return np.float32(np.mean(cham) / 12.8)


# revision 4
# speedup vs baseline: 1.0091x; 1.0091x over previous
"""Chamfer distance loss kernel v2 for Trainium2 (8 NeuronCores, Bass/Tile).

Problem: A, B [4, 8192, 3] f32 point clouds ->
    mean_b( mean_n min_m ||A[b,n]-B[b,m]|| + mean_m min_n ||.|| ) / 12.8

Strategy (per core = one batch x one half of A's rows):
  - [4096 x 8192] squared-distance block via K=13 float32r matmuls
    (11-bit hi/lo split => fp32-exact d^2 at 1 PE cycle/row).
  - All consumption uses the negated convention (-d^2, MAX = min of d^2).
  - Per row tile (128 rows x 8192 cols = 4 PSUM groups of 2048): ACT stages
    each group to f16 (scale=-1) — ACT is the only cheap PSUM reader
    (0.83 ns/elem); everything else must run on DVE since Pool/gpsimd
    cannot execute tensor ops and DMA cannot accumulate min/max or read
    PSUM on this toolchain.
  - DVE consumes the staged f16 tile at 2x (0.52 ns/elem) with max-width
    ops: one 8192-wide tensor_tensor MAX into persistent bmin [128, 8192]
    (B-side col-min), then a 3-op fold chain 8192->4096->2048->1024 into
    amin_w [128, 32, 1024] (A-side row-min partials).
  - No on-device cross-partition reduction: bmin ships raw, amin_w ships
    progressively in 8 chunks; the host does the cheap tails: fold
    1024-wide slots, min across 128 partitions, core/batch combine,
    clamp/sqrt/means. min/sqrt/mean commute with the sharding.
  - Engine busy/row-tile: DVE 8.6us (bottleneck), ACT 7.9us, PE 3.4-6.8us.
"""
import os
import hashlib
import shutil
import numpy as np
from contextlib import ExitStack

import concourse.bass as bass
import concourse.tile as tile
import concourse.mybir as mybir
import concourse.bass2jax as bass2jax
from concourse import bass_utils
from concourse.vector_clock import ScopedClock

# ---------------------------------------------------------------------------
# Patch 1: walrus encodes at most ONE sync wait per TPB instruction
# ("Too many sync wait commands"). Tile attaches several (incl. the tail
# drain). Split extras onto preceding same-engine EventSemaphore/Drain
# instructions.
# ---------------------------------------------------------------------------


def _patched_drain_and_barrier(self, tick_clock, wait_clock):
    nc = self.nc
    drain_inst = nc.sync.drain()
    wait_clock.add_sem_waits(
        drain_inst.ins, ScopedClock({None: tick_clock.global_clock})
    )
    si = drain_inst.ins.sync_info
    if si is not None and len(si.on_wait) > 1:
        waits = list(si.on_wait)
        drain_inst.ins.sync_info = mybir.SyncInfo(
            on_wait=waits[:1], on_update=list(si.on_update)
        )
        for i in range(1, len(waits)):
            extra = nc.sync.drain()
            extra.ins.sync_info = mybir.SyncInfo(
                on_wait=waits[i:i + 1], on_update=[]
            )

    nc.all_engine_barrier()
    assert self.sems is not None
    popped = nc._tile_sem_poison_stack.pop()
    assert popped is self._sem_poison
    nc.clear_and_free_semaphores(list(self.sems.allocated().values()))
    nc.all_engine_barrier()


tile.TileContext._drain_and_barrier = _patched_drain_and_barrier

_split_counter = [0]


def _split_multi_waits(nc):
    for f in nc.m.functions:
        for bb in f.blocks:
            insts = bb.instructions
            out = []
            changed = False
            for inst in insts:
                si = inst.sync_info
                if si is not None and len(si.on_wait) > 1:
                    waits = list(si.on_wait)
                    for w in waits[:-1]:
                        _split_counter[0] += 1
                        ev = mybir.InstEventSemaphore(
                            name=f"evsplit_{_split_counter[0]}"
                        )
                        ev.engine = inst.engine
                        ev.sync_info = mybir.SyncInfo(on_wait=[w], on_update=[])
                        out.append(ev)
                    inst.sync_info = mybir.SyncInfo(
                        on_wait=waits[-1:], on_update=list(si.on_update)
                    )
                    changed = True
                out.append(inst)
            if changed:
                bb.instructions = out


# ---------------------------------------------------------------------------
# Patch 2: disk-cache compiled NEFFs by BIR hash so repeated kernel() calls
# and processes skip the multi-minute walrus compile.
# ---------------------------------------------------------------------------

_NEFF_CACHE_DIR = os.environ.get("BASS_NEFF_CACHE_DIR", "/tmp/bass_neff_cache")
_orig_compile_bir_kernel = bass_utils.compile_bir_kernel


def _cached_compile_bir_kernel(bir_json, tmpdir, neff_name="file.neff"):
    try:
        os.makedirs(_NEFF_CACHE_DIR, exist_ok=True)
        key = hashlib.sha256(bir_json).hexdigest()
        cpath = os.path.join(_NEFF_CACHE_DIR, f"{key}_{neff_name}")
        dst_dir = os.path.join(tmpdir, "sg00")
        dst = os.path.join(dst_dir, neff_name)
        if os.path.exists(cpath):
            os.makedirs(dst_dir, exist_ok=True)
            shutil.copyfile(cpath, dst)
            return dst
        out = _orig_compile_bir_kernel(bir_json, tmpdir, neff_name)
        try:
            shutil.copyfile(out, cpath)
        except OSError:
            pass
        return out
    except Exception:
        return _orig_compile_bir_kernel(bir_json, tmpdir, neff_name)


bass_utils.compile_bir_kernel = _cached_compile_bir_kernel
bass2jax.compile_bir_kernel = _cached_compile_bir_kernel

# ---------------------------------------------------------------------------
# Kernel build
# ---------------------------------------------------------------------------

F16 = mybir.dt.float16
F32 = mybir.dt.float32
F32R = mybir.dt.float32r
MIN = mybir.AluOpType.min
MAX = mybir.AluOpType.max
COPYF = mybir.ActivationFunctionType.Copy

KK = 13        # hi/lo-split augmented contraction dim
P = 128
CHUNK = 512    # PSUM bank free size (fp32)
GW = 2048      # group width (4 banks)
NG = 4         # groups per row tile
BATCH = 4
N = 8192
HALF = N // 2
RT = HALF // P  # 32 row tiles
N_CORES = 8
SPLIT_BITS = 11
NEG_INIT = -60000.0  # f16-representable, below any -d^2

def _build_nc():
    nc = bass.Bass(trn_type="TRN2")
    # declared f32r (same bits as f32) so the HWDGE engines can load the
    # inputs without the gpsimd cast path serializing the warmup
    lhsT_d = nc.dram_tensor("lhsT", [KK, HALF], F32R, kind="ExternalInput")
    rhsB_d = nc.dram_tensor("rhsB", [KK, N], F32R, kind="ExternalInput")
    aw_d = nc.dram_tensor("aw", [P, RT, 1024], F16, kind="ExternalOutput")
    bmin_d = nc.dram_tensor("bmin", [P, N], F16, kind="ExternalOutput")

    with tile.TileContext(nc) as tc:
        with ExitStack() as ctx:
            consts = ctx.enter_context(tc.tile_pool(name="consts", bufs=1))
            psum = ctx.enter_context(
                tc.tile_pool(name="psum", bufs=2, space="PSUM")
            )
            tpool = ctx.enter_context(tc.tile_pool(name="tpool", bufs=2))
            scr = ctx.enter_context(tc.tile_pool(name="scr", bufs=2))

            lhs_sb = consts.tile([KK, HALF], F32R)
            nc.sync.dma_start(out=lhs_sb, in_=lhsT_d[:, :])
            rhs_sb = consts.tile([KK, N], F32R)
            for g0 in range(4):
                nc.sync.dma_start(
                    out=rhs_sb[:, g0 * 2048:(g0 + 1) * 2048],
                    in_=rhsB_d[:, g0 * 2048:(g0 + 1) * 2048],
                )

            bmin = consts.tile([P, N], F16)
            # init bmin below any -d^2 value; per-group owners then MAX in.
            nc.vector.memset(bmin, NEG_INIT)
            amin_w = consts.tile([P, RT, 1024], F16)

            for i in range(RT):
                T = tpool.tile([P, N], F16, tag="T")
                for g in range(NG):
                    pt = psum.tile([P, GW], F32, tag="pt")
                    for q in range(NG):
                        j = g * NG + q
                        nc.tensor.matmul(
                            pt[:, q * CHUNK:(q + 1) * CHUNK],
                            lhs_sb[:, i * P:(i + 1) * P],
                            rhs_sb[:, j * CHUNK:(j + 1) * CHUNK],
                            start=True,
                            stop=True,
                        )
                    nc.scalar.activation(
                        out=T[:, g * GW:(g + 1) * GW], in_=pt,
                        func=COPYF, scale=-1.0,
                    )
                # B-side: one full-width accumulate (col-min as MAX of -d^2)
                nc.vector.tensor_tensor(out=bmin, in0=T, in1=bmin, op=MAX)
                # A-side fold chain to 1024-wide slots
                c1 = scr.tile([P, N // 2], F16, tag="c1")
                nc.vector.tensor_tensor(
                    out=c1, in0=T[:, 0:N // 2], in1=T[:, N // 2:N], op=MAX
                )
                c2 = scr.tile([P, GW], F16, tag="c2")
                nc.vector.tensor_tensor(
                    out=c2, in0=c1[:, 0:GW], in1=c1[:, GW:2 * GW], op=MAX
                )
                nc.vector.tensor_tensor(
                    out=amin_w[:, i, :], in0=c2[:, 0:1024], in1=c2[:, 1024:2048],
                    op=MAX,
                )
                # ship amin_w progressively in 8 chunks of 4 tiles
                if i % 4 == 3:
                    c = i // 4
                    nc.sync.dma_start(
                        out=aw_d[:, c * 4:(c + 1) * 4, :],
                        in_=amin_w[:, c * 4:(c + 1) * 4, :],
                    )
            nc.sync.dma_start(out=bmin_d[:, :], in_=bmin)
    _split_multi_waits(nc)
    return nc


_NC = None


def _get_nc():
    global _NC
    if _NC is None:
        _NC = _build_nc()
    return _NC


def _round_mant(v, bits=SPLIT_BITS):
    m, e = np.frexp(v.astype(np.float64))
    return np.ldexp(np.round(m * (1 << bits)) / (1 << bits), e).astype(np.float32)


def _host_prep_core(Asub, Bfull):
    """Build the K=13 hi/lo-split augmented operands (all 11-bit exact)."""
    a2 = (Asub.astype(np.float32) ** 2).sum(axis=1)
    b2 = (Bfull.astype(np.float32) ** 2).sum(axis=1)
    ah = _round_mant(Asub.T)
    al = (Asub.T - ah).astype(np.float32)
    bh = _round_mant(Bfull.T)
    bl = (Bfull.T - bh).astype(np.float32)
    a2h = _round_mant(a2)
    a2l = (a2 - a2h).astype(np.float32)
    b2h = _round_mant(b2)
    b2l = (b2 - b2h).astype(np.float32)

    lhsT = np.empty((KK, Asub.shape[0]), np.float32)
    rhsB = np.empty((KK, Bfull.shape[0]), np.float32)
    lhsT[0:3] = ah
    rhsB[0:3] = -2.0 * bh
    lhsT[3:6] = ah
    rhsB[3:6] = -2.0 * bl
    lhsT[6:9] = al
    rhsB[6:9] = -2.0 * bh
    lhsT[9] = a2h
    rhsB[9] = 1.0
    lhsT[10] = a2l
    rhsB[10] = 1.0
    lhsT[11] = 1.0
    rhsB[11] = b2h
    lhsT[12] = 1.0
    rhsB[12] = b2l
    return {"lhsT": lhsT, "rhsB": rhsB}


def kernel(A, B):
    A = np.ascontiguousarray(np.asarray(A, dtype=np.float32))
    B = np.ascontiguousarray(np.asarray(B, dtype=np.float32))
    nc = _get_nc()

    in_maps = []
    for c in range(N_CORES):
        b, h = divmod(c, 2)
        in_maps.append(_host_prep_core(A[b, h * HALF:(h + 1) * HALF], B[b]))

    res = bass_utils.run_bass_kernel_spmd(
        nc, in_maps, core_ids=list(range(N_CORES))
    )

    cham = []
    for b in range(BATCH):
        a_rows = []   # min d^2 per A row
        b_sq = None   # columnwise min d^2 over all rows
        for h in range(2):
            r = res.results[2 * b + h]
            aw = np.asarray(r["aw"], dtype=np.float32)      # [128, 32, 1024] (-d^2)
            bm = np.asarray(r["bmin"], dtype=np.float32)    # [128, 8192] (-d^2)
            a_d2 = -aw.max(axis=2)                          # [128, 32]
            # row index = i*128 + p  ->  [32, 128] -> flat
            a_rows.append(a_d2.T.reshape(-1))
            cb = -bm.max(axis=0)                            # [8192]
            b_sq = cb if b_sq is None else np.minimum(b_sq, cb)
        a_sq = np.concatenate(a_rows)
        da = np.sqrt(np.maximum(a_sq, 0.0))
        db = np.sqrt(np.maximum(b_sq, 0.0))
        cham.append(da.mean() + db.mean())

    return np.float32(np.mean(cham) / 12.8)


# revision 6
# speedup vs baseline: 1.0933x; 1.0835x over previous
"""Chamfer distance loss kernel v2 for Trainium2 (8 NeuronCores, Bass/Tile).

Problem: A, B [4, 8192, 3] f32 point clouds ->
    mean_b( mean_n min_m ||A[b,n]-B[b,m]|| + mean_m min_n ||.|| ) / 12.8

Strategy (per core = one batch x one half of A's rows):
  - [4096 x 8192] squared-distance block via K=13 float32r matmuls
    (11-bit hi/lo split => fp32-exact d^2 at 1 PE cycle/row).
  - All consumption uses the negated convention (-d^2, MAX = min of d^2).
  - Per row tile (128 rows x 8192 cols = 4 PSUM groups of 2048): ACT stages
    each group to f16 (scale=-1) — ACT is the only cheap PSUM reader
    (0.83 ns/elem); everything else must run on DVE since Pool/gpsimd
    cannot execute tensor ops and DMA cannot accumulate min/max or read
    PSUM on this toolchain.
  - DVE consumes the staged f16 tile at 2x (0.52 ns/elem) with max-width
    ops: one 8192-wide tensor_tensor MAX into persistent bmin [128, 8192]
    (B-side col-min) plus ONE 8192->4096 fold (A-side); the 4096-wide
    row-min partials ship to DRAM per row tile on the otherwise-idle DMA
    path, keeping DVE at 6.7us/tile under ACT's 7.9us staging floor.
  - Host does the cheap tails: fold the 4096-wide slots, min across 128
    partitions, core/batch combine, clamp/sqrt/means — all commute with
    the sharding.
  - Engine busy/row-tile: ACT 7.9us (bottleneck), DVE 6.7us, PE 3.4-6.8us.
"""
import os
import hashlib
import shutil
import numpy as np
from contextlib import ExitStack

import concourse.bass as bass
import concourse.tile as tile
import concourse.mybir as mybir
import concourse.bass2jax as bass2jax
from concourse import bass_utils
from concourse.vector_clock import ScopedClock

# ---------------------------------------------------------------------------
# Patch 1: walrus encodes at most ONE sync wait per TPB instruction
# ("Too many sync wait commands"). Tile attaches several (incl. the tail
# drain). Split extras onto preceding same-engine EventSemaphore/Drain
# instructions.
# ---------------------------------------------------------------------------


def _patched_drain_and_barrier(self, tick_clock, wait_clock):
    nc = self.nc
    drain_inst = nc.sync.drain()
    wait_clock.add_sem_waits(
        drain_inst.ins, ScopedClock({None: tick_clock.global_clock})
    )
    si = drain_inst.ins.sync_info
    if si is not None and len(si.on_wait) > 1:
        waits = list(si.on_wait)
        drain_inst.ins.sync_info = mybir.SyncInfo(
            on_wait=waits[:1], on_update=list(si.on_update)
        )
        for i in range(1, len(waits)):
            extra = nc.sync.drain()
            extra.ins.sync_info = mybir.SyncInfo(
                on_wait=waits[i:i + 1], on_update=[]
            )

    nc.all_engine_barrier()
    assert self.sems is not None
    popped = nc._tile_sem_poison_stack.pop()
    assert popped is self._sem_poison
    nc.clear_and_free_semaphores(list(self.sems.allocated().values()))
    nc.all_engine_barrier()


tile.TileContext._drain_and_barrier = _patched_drain_and_barrier

_split_counter = [0]


def _split_multi_waits(nc):
    for f in nc.m.functions:
        for bb in f.blocks:
            insts = bb.instructions
            out = []
            changed = False
            for inst in insts:
                si = inst.sync_info
                if si is not None and len(si.on_wait) > 1:
                    waits = list(si.on_wait)
                    for w in waits[:-1]:
                        _split_counter[0] += 1
                        ev = mybir.InstEventSemaphore(
                            name=f"evsplit_{_split_counter[0]}"
                        )
                        ev.engine = inst.engine
                        ev.sync_info = mybir.SyncInfo(on_wait=[w], on_update=[])
                        out.append(ev)
                    inst.sync_info = mybir.SyncInfo(
                        on_wait=waits[-1:], on_update=list(si.on_update)
                    )
                    changed = True
                out.append(inst)
            if changed:
                bb.instructions = out


# ---------------------------------------------------------------------------
# Patch 2: disk-cache compiled NEFFs by BIR hash so repeated kernel() calls
# and processes skip the multi-minute walrus compile.
# ---------------------------------------------------------------------------

_NEFF_CACHE_DIR = os.environ.get("BASS_NEFF_CACHE_DIR", "/tmp/bass_neff_cache")
_orig_compile_bir_kernel = bass_utils.compile_bir_kernel


def _cached_compile_bir_kernel(bir_json, tmpdir, neff_name="file.neff"):
    try:
        os.makedirs(_NEFF_CACHE_DIR, exist_ok=True)
        key = hashlib.sha256(bir_json).hexdigest()
        cpath = os.path.join(_NEFF_CACHE_DIR, f"{key}_{neff_name}")
        dst_dir = os.path.join(tmpdir, "sg00")
        dst = os.path.join(dst_dir, neff_name)
        if os.path.exists(cpath):
            os.makedirs(dst_dir, exist_ok=True)
            shutil.copyfile(cpath, dst)
            return dst
        out = _orig_compile_bir_kernel(bir_json, tmpdir, neff_name)
        try:
            shutil.copyfile(out, cpath)
        except OSError:
            pass
        return out
    except Exception:
        return _orig_compile_bir_kernel(bir_json, tmpdir, neff_name)


bass_utils.compile_bir_kernel = _cached_compile_bir_kernel
bass2jax.compile_bir_kernel = _cached_compile_bir_kernel

# ---------------------------------------------------------------------------
# Kernel build
# ---------------------------------------------------------------------------

F16 = mybir.dt.float16
F32 = mybir.dt.float32
F32R = mybir.dt.float32r
MIN = mybir.AluOpType.min
MAX = mybir.AluOpType.max
COPYF = mybir.ActivationFunctionType.Copy

KK = 13        # hi/lo-split augmented contraction dim
P = 128
CHUNK = 512    # PSUM bank free size (fp32)
GW = 2048      # group width (4 banks)
NG = 4         # groups per row tile
BATCH = 4
N = 8192
HALF = N // 2
RT = HALF // P  # 32 row tiles
N_CORES = 8
SPLIT_BITS = 11
NEG_INIT = -60000.0  # f16-representable, below any -d^2

def _build_nc():
    nc = bass.Bass(trn_type="TRN2")
    # declared f32r (same bits as f32) so the HWDGE engines can load the
    # inputs without the gpsimd cast path serializing the warmup
    lhsT_d = nc.dram_tensor("lhsT", [KK, HALF], F32R, kind="ExternalInput")
    rhsB_d = nc.dram_tensor("rhsB", [KK, N], F32R, kind="ExternalInput")
    aw_d = nc.dram_tensor("aw", [P, RT, 4096], F16, kind="ExternalOutput")
    bmin_d = nc.dram_tensor("bmin", [P, N], F16, kind="ExternalOutput")

    with tile.TileContext(nc) as tc:
        with ExitStack() as ctx:
            consts = ctx.enter_context(tc.tile_pool(name="consts", bufs=1))
            psum = ctx.enter_context(
                tc.tile_pool(name="psum", bufs=2, space="PSUM")
            )
            tpool = ctx.enter_context(tc.tile_pool(name="tpool", bufs=2))
            scr = ctx.enter_context(tc.tile_pool(name="scr", bufs=2))

            lhs_sb = consts.tile([KK, HALF], F32R)
            nc.sync.dma_start(out=lhs_sb, in_=lhsT_d[:, :])
            rhs_sb = consts.tile([KK, N], F32R)
            for g0 in range(4):
                nc.sync.dma_start(
                    out=rhs_sb[:, g0 * 2048:(g0 + 1) * 2048],
                    in_=rhsB_d[:, g0 * 2048:(g0 + 1) * 2048],
                )

            bmin = consts.tile([P, N], F16)
            # init bmin below any -d^2 value; per-group owners then MAX in.
            nc.vector.memset(bmin, NEG_INIT)

            for i in range(RT):
                T = tpool.tile([P, N], F16, tag="T")
                for g in range(NG):
                    pt = psum.tile([P, GW], F32, tag="pt")
                    for q in range(NG):
                        j = g * NG + q
                        nc.tensor.matmul(
                            pt[:, q * CHUNK:(q + 1) * CHUNK],
                            lhs_sb[:, i * P:(i + 1) * P],
                            rhs_sb[:, j * CHUNK:(j + 1) * CHUNK],
                            start=True,
                            stop=True,
                        )
                    nc.scalar.activation(
                        out=T[:, g * GW:(g + 1) * GW], in_=pt,
                        func=COPYF, scale=-1.0,
                    )
                # B-side: one full-width accumulate (col-min as MAX of -d^2);
                # last tile splits in half so bmin's output DMA overlaps
                if i == RT - 1:
                    nc.vector.tensor_tensor(
                        out=bmin[:, 0:4096], in0=T[:, 0:4096],
                        in1=bmin[:, 0:4096], op=MAX,
                    )
                    nc.sync.dma_start(out=bmin_d[:, 0:4096], in_=bmin[:, 0:4096])
                    nc.vector.tensor_tensor(
                        out=bmin[:, 4096:8192], in0=T[:, 4096:8192],
                        in1=bmin[:, 4096:8192], op=MAX,
                    )
                    nc.sync.dma_start(
                        out=bmin_d[:, 4096:8192], in_=bmin[:, 4096:8192]
                    )
                else:
                    nc.vector.tensor_tensor(out=bmin, in0=T, in1=bmin, op=MAX)
                # A-side: single on-device fold 8192->4096; the rest of the
                # row-min happens on the host (DMA + host are idle, DVE is
                # the bottleneck engine)
                c1 = scr.tile([P, N // 2], F16, tag="c1")
                nc.vector.tensor_tensor(
                    out=c1, in0=T[:, 0:N // 2], in1=T[:, N // 2:N], op=MAX
                )
                nc.sync.dma_start(out=aw_d[:, i, :], in_=c1)
    _split_multi_waits(nc)
    return nc


_NC = None


def _get_nc():
    global _NC
    if _NC is None:
        _NC = _build_nc()
    return _NC


def _round_mant(v, bits=SPLIT_BITS):
    m, e = np.frexp(v.astype(np.float64))
    return np.ldexp(np.round(m * (1 << bits)) / (1 << bits), e).astype(np.float32)


def _host_prep_core(Asub, Bfull):
    """Build the K=13 hi/lo-split augmented operands (all 11-bit exact)."""
    a2 = (Asub.astype(np.float32) ** 2).sum(axis=1)
    b2 = (Bfull.astype(np.float32) ** 2).sum(axis=1)
    ah = _round_mant(Asub.T)
    al = (Asub.T - ah).astype(np.float32)
    bh = _round_mant(Bfull.T)
    bl = (Bfull.T - bh).astype(np.float32)
    a2h = _round_mant(a2)
    a2l = (a2 - a2h).astype(np.float32)
    b2h = _round_mant(b2)
    b2l = (b2 - b2h).astype(np.float32)

    lhsT = np.empty((KK, Asub.shape[0]), np.float32)
    rhsB = np.empty((KK, Bfull.shape[0]), np.float32)
    lhsT[0:3] = ah
    rhsB[0:3] = -2.0 * bh
    lhsT[3:6] = ah
    rhsB[3:6] = -2.0 * bl
    lhsT[6:9] = al
    rhsB[6:9] = -2.0 * bh
    lhsT[9] = a2h
    rhsB[9] = 1.0
    lhsT[10] = a2l
    rhsB[10] = 1.0
    lhsT[11] = 1.0
    rhsB[11] = b2h
    lhsT[12] = 1.0
    rhsB[12] = b2l
    return {"lhsT": lhsT, "rhsB": rhsB}


def kernel(A, B):
    A = np.ascontiguousarray(np.asarray(A, dtype=np.float32))
    B = np.ascontiguousarray(np.asarray(B, dtype=np.float32))
    nc = _get_nc()

    in_maps = []
    for c in range(N_CORES):
        b, h = divmod(c, 2)
        in_maps.append(_host_prep_core(A[b, h * HALF:(h + 1) * HALF], B[b]))

    res = bass_utils.run_bass_kernel_spmd(
        nc, in_maps, core_ids=list(range(N_CORES))
    )

    cham = []
    for b in range(BATCH):
        a_rows = []   # min d^2 per A row
        b_sq = None   # columnwise min d^2 over all rows
        for h in range(2):
            r = res.results[2 * b + h]
            aw = np.asarray(r["aw"], dtype=np.float32)      # [128, 32, 4096] (-d^2)
            bm = np.asarray(r["bmin"], dtype=np.float32)    # [128, 8192] (-d^2)
            a_d2 = -aw.max(axis=2)                          # [128, 32]
            # row index = i*128 + p  ->  [32, 128] -> flat
            a_rows.append(a_d2.T.reshape(-1))
            cb = -bm.max(axis=0)                            # [8192]
            b_sq = cb if b_sq is None else np.minimum(b_sq, cb)
        a_sq = np.concatenate(a_rows)
        da = np.sqrt(np.maximum(a_sq, 0.0))
        db = np.sqrt(np.maximum(b_sq, 0.0))
        cham.append(da.mean() + db.mean())

    return np.float32(np.mean(cham) / 12.8)


# revision 7
# speedup vs baseline: 1.1071x; 1.0126x over previous
"""Chamfer distance loss kernel v2 for Trainium2 (8 NeuronCores, Bass/Tile).

Problem: A, B [4, 8192, 3] f32 point clouds ->
    mean_b( mean_n min_m ||A[b,n]-B[b,m]|| + mean_m min_n ||.|| ) / 12.8

Strategy (per core = one batch x one half of A's rows):
  - [4096 x 8192] squared-distance block via K=13 float32r matmuls
    (11-bit hi/lo split => fp32-exact d^2 at 1 PE cycle/row).
  - All consumption uses the negated convention (-d^2, MAX = min of d^2).
  - Per row tile (128 rows x 8192 cols = 4 PSUM groups of 2048): ACT stages
    each group to f16 (scale=-1) — ACT is the only cheap PSUM reader
    (0.83 ns/elem); everything else must run on DVE since Pool/gpsimd
    cannot execute tensor ops and DMA cannot accumulate min/max or read
    PSUM on this toolchain.
  - DVE consumes the staged f16 tile at 2x (0.52 ns/elem) with max-width
    ops: one 8192-wide tensor_tensor MAX into persistent bmin [128, 8192]
    (B-side col-min) plus ONE 8192->4096 fold (A-side); the 4096-wide
    row-min partials ship to DRAM per row tile on the otherwise-idle DMA
    path, keeping DVE at 6.7us/tile under ACT's 7.9us staging floor.
  - Host does the cheap tails: fold the 4096-wide slots, min across 128
    partitions, core/batch combine, clamp/sqrt/means — all commute with
    the sharding.
  - Engine busy/row-tile: ACT 7.9us (bottleneck), DVE 6.7us, PE 3.4-6.8us.
"""
import os
import hashlib
import shutil
import numpy as np
from contextlib import ExitStack

import concourse.bass as bass
import concourse.tile as tile
import concourse.mybir as mybir
import concourse.bass2jax as bass2jax
from concourse import bass_utils
from concourse.vector_clock import ScopedClock

# ---------------------------------------------------------------------------
# Patch 1: walrus encodes at most ONE sync wait per TPB instruction
# ("Too many sync wait commands"). Tile attaches several (incl. the tail
# drain). Split extras onto preceding same-engine EventSemaphore/Drain
# instructions.
# ---------------------------------------------------------------------------


def _patched_drain_and_barrier(self, tick_clock, wait_clock):
    nc = self.nc
    drain_inst = nc.sync.drain()
    wait_clock.add_sem_waits(
        drain_inst.ins, ScopedClock({None: tick_clock.global_clock})
    )
    si = drain_inst.ins.sync_info
    if si is not None and len(si.on_wait) > 1:
        waits = list(si.on_wait)
        drain_inst.ins.sync_info = mybir.SyncInfo(
            on_wait=waits[:1], on_update=list(si.on_update)
        )
        for i in range(1, len(waits)):
            extra = nc.sync.drain()
            extra.ins.sync_info = mybir.SyncInfo(
                on_wait=waits[i:i + 1], on_update=[]
            )

    nc.all_engine_barrier()
    assert self.sems is not None
    popped = nc._tile_sem_poison_stack.pop()
    assert popped is self._sem_poison
    nc.clear_and_free_semaphores(list(self.sems.allocated().values()))
    nc.all_engine_barrier()


tile.TileContext._drain_and_barrier = _patched_drain_and_barrier

_split_counter = [0]


def _split_multi_waits(nc):
    for f in nc.m.functions:
        for bb in f.blocks:
            insts = bb.instructions
            out = []
            changed = False
            for inst in insts:
                si = inst.sync_info
                if si is not None and len(si.on_wait) > 1:
                    waits = list(si.on_wait)
                    for w in waits[:-1]:
                        _split_counter[0] += 1
                        ev = mybir.InstEventSemaphore(
                            name=f"evsplit_{_split_counter[0]}"
                        )
                        ev.engine = inst.engine
                        ev.sync_info = mybir.SyncInfo(on_wait=[w], on_update=[])
                        out.append(ev)
                    inst.sync_info = mybir.SyncInfo(
                        on_wait=waits[-1:], on_update=list(si.on_update)
                    )
                    changed = True
                out.append(inst)
            if changed:
                bb.instructions = out


# ---------------------------------------------------------------------------
# Patch 2: disk-cache compiled NEFFs by BIR hash so repeated kernel() calls
# and processes skip the multi-minute walrus compile.
# ---------------------------------------------------------------------------

_NEFF_CACHE_DIR = os.environ.get("BASS_NEFF_CACHE_DIR", "/tmp/bass_neff_cache")
_orig_compile_bir_kernel = bass_utils.compile_bir_kernel


def _cached_compile_bir_kernel(bir_json, tmpdir, neff_name="file.neff"):
    try:
        os.makedirs(_NEFF_CACHE_DIR, exist_ok=True)
        key = hashlib.sha256(bir_json).hexdigest()
        cpath = os.path.join(_NEFF_CACHE_DIR, f"{key}_{neff_name}")
        dst_dir = os.path.join(tmpdir, "sg00")
        dst = os.path.join(dst_dir, neff_name)
        if os.path.exists(cpath):
            os.makedirs(dst_dir, exist_ok=True)
            shutil.copyfile(cpath, dst)
            return dst
        out = _orig_compile_bir_kernel(bir_json, tmpdir, neff_name)
        try:
            shutil.copyfile(out, cpath)
        except OSError:
            pass
        return out
    except Exception:
        return _orig_compile_bir_kernel(bir_json, tmpdir, neff_name)


bass_utils.compile_bir_kernel = _cached_compile_bir_kernel
bass2jax.compile_bir_kernel = _cached_compile_bir_kernel

# ---------------------------------------------------------------------------
# Kernel build
# ---------------------------------------------------------------------------

F16 = mybir.dt.float16
F32 = mybir.dt.float32
F32R = mybir.dt.float32r
MIN = mybir.AluOpType.min
MAX = mybir.AluOpType.max
COPYF = mybir.ActivationFunctionType.Copy

KK = 13        # hi/lo-split augmented contraction dim
P = 128
CHUNK = 512    # PSUM bank free size (fp32)
GW = 2048      # group width (4 banks)
NG = 4         # groups per row tile
BATCH = 4
N = 8192
HALF = N // 2
RT = HALF // P  # 32 row tiles
N_CORES = 8
SPLIT_BITS = 11
NEG_INIT = -60000.0  # f16-representable, below any -d^2

def _build_nc():
    nc = bass.Bass(trn_type="TRN2")
    # declared f32r (same bits as f32) so the HWDGE engines can load the
    # inputs without the gpsimd cast path serializing the warmup
    lhsT_d = nc.dram_tensor("lhsT", [KK, HALF], F32R, kind="ExternalInput")
    rhsB_d = nc.dram_tensor("rhsB", [KK, N], F32R, kind="ExternalInput")
    aw_d = nc.dram_tensor("aw", [P, RT, 4096], F16, kind="ExternalOutput")
    bmin_d = nc.dram_tensor("bmin", [P, N], F16, kind="ExternalOutput")

    with tile.TileContext(nc) as tc:
        with ExitStack() as ctx:
            consts = ctx.enter_context(tc.tile_pool(name="consts", bufs=1))
            psum = ctx.enter_context(
                tc.tile_pool(name="psum", bufs=2, space="PSUM")
            )
            tpool = ctx.enter_context(tc.tile_pool(name="tpool", bufs=2))
            scr = ctx.enter_context(tc.tile_pool(name="scr", bufs=2))

            lhs_sb = consts.tile([KK, HALF], F32R)
            nc.sync.dma_start(out=lhs_sb, in_=lhsT_d[:, :])
            rhs_sb = consts.tile([KK, N], F32R)
            for g0 in range(4):
                nc.sync.dma_start(
                    out=rhs_sb[:, g0 * 2048:(g0 + 1) * 2048],
                    in_=rhsB_d[:, g0 * 2048:(g0 + 1) * 2048],
                )

            bmin = consts.tile([P, N], F16)
            # init bmin below any -d^2 value; per-group owners then MAX in.
            nc.vector.memset(bmin, NEG_INIT)

            for i in range(RT):
                T = tpool.tile([P, N], F16, tag="T")
                for g in range(NG):
                    pt = psum.tile([P, GW], F32, tag="pt")
                    for q in range(NG):
                        j = g * NG + q
                        nc.tensor.matmul(
                            pt[:, q * CHUNK:(q + 1) * CHUNK],
                            lhs_sb[:, i * P:(i + 1) * P],
                            rhs_sb[:, j * CHUNK:(j + 1) * CHUNK],
                            start=True,
                            stop=True,
                        )
                    if g == NG - 1:
                        # rebalance: ACT is the bottleneck engine, DVE has
                        # slack — DVE stages the last 512 columns (negating
                        # via tensor_scalar mult)
                        nc.scalar.activation(
                            out=T[:, g * GW:g * GW + 1536], in_=pt[:, 0:1536],
                            func=COPYF, scale=-1.0,
                        )
                        nc.vector.tensor_scalar_mul(
                            T[:, g * GW + 1536:(g + 1) * GW],
                            pt[:, 1536:2048], -1.0,
                        )
                    else:
                        nc.scalar.activation(
                            out=T[:, g * GW:(g + 1) * GW], in_=pt,
                            func=COPYF, scale=-1.0,
                        )
                # B-side: one full-width accumulate (col-min as MAX of -d^2);
                # last tile splits in half so bmin's output DMA overlaps
                if i == RT - 1:
                    nc.vector.tensor_tensor(
                        out=bmin[:, 0:4096], in0=T[:, 0:4096],
                        in1=bmin[:, 0:4096], op=MAX,
                    )
                    nc.sync.dma_start(out=bmin_d[:, 0:4096], in_=bmin[:, 0:4096])
                    nc.vector.tensor_tensor(
                        out=bmin[:, 4096:8192], in0=T[:, 4096:8192],
                        in1=bmin[:, 4096:8192], op=MAX,
                    )
                    nc.sync.dma_start(
                        out=bmin_d[:, 4096:8192], in_=bmin[:, 4096:8192]
                    )
                else:
                    nc.vector.tensor_tensor(out=bmin, in0=T, in1=bmin, op=MAX)
                # A-side: single on-device fold 8192->4096; the rest of the
                # row-min happens on the host (DMA + host are idle, DVE is
                # the bottleneck engine)
                c1 = scr.tile([P, N // 2], F16, tag="c1")
                nc.vector.tensor_tensor(
                    out=c1, in0=T[:, 0:N // 2], in1=T[:, N // 2:N], op=MAX
                )
                nc.sync.dma_start(out=aw_d[:, i, :], in_=c1)
    _split_multi_waits(nc)
    return nc


_NC = None


def _get_nc():
    global _NC
    if _NC is None:
        _NC = _build_nc()
    return _NC


def _round_mant(v, bits=SPLIT_BITS):
    m, e = np.frexp(v.astype(np.float64))
    return np.ldexp(np.round(m * (1 << bits)) / (1 << bits), e).astype(np.float32)


def _host_prep_core(Asub, Bfull):
    """Build the K=13 hi/lo-split augmented operands (all 11-bit exact)."""
    a2 = (Asub.astype(np.float32) ** 2).sum(axis=1)
    b2 = (Bfull.astype(np.float32) ** 2).sum(axis=1)
    ah = _round_mant(Asub.T)
    al = (Asub.T - ah).astype(np.float32)
    bh = _round_mant(Bfull.T)
    bl = (Bfull.T - bh).astype(np.float32)
    a2h = _round_mant(a2)
    a2l = (a2 - a2h).astype(np.float32)
    b2h = _round_mant(b2)
    b2l = (b2 - b2h).astype(np.float32)

    lhsT = np.empty((KK, Asub.shape[0]), np.float32)
    rhsB = np.empty((KK, Bfull.shape[0]), np.float32)
    lhsT[0:3] = ah
    rhsB[0:3] = -2.0 * bh
    lhsT[3:6] = ah
    rhsB[3:6] = -2.0 * bl
    lhsT[6:9] = al
    rhsB[6:9] = -2.0 * bh
    lhsT[9] = a2h
    rhsB[9] = 1.0
    lhsT[10] = a2l
    rhsB[10] = 1.0
    lhsT[11] = 1.0
    rhsB[11] = b2h
    lhsT[12] = 1.0
    rhsB[12] = b2l
    return {"lhsT": lhsT, "rhsB": rhsB}


def kernel(A, B):
    A = np.ascontiguousarray(np.asarray(A, dtype=np.float32))
    B = np.ascontiguousarray(np.asarray(B, dtype=np.float32))
    nc = _get_nc()

    in_maps = []
    for c in range(N_CORES):
        b, h = divmod(c, 2)
        in_maps.append(_host_prep_core(A[b, h * HALF:(h + 1) * HALF], B[b]))

    res = bass_utils.run_bass_kernel_spmd(
        nc, in_maps, core_ids=list(range(N_CORES))
    )

    cham = []
    for b in range(BATCH):
        a_rows = []   # min d^2 per A row
        b_sq = None   # columnwise min d^2 over all rows
        for h in range(2):
            r = res.results[2 * b + h]
            aw = np.asarray(r["aw"], dtype=np.float32)      # [128, 32, 4096] (-d^2)
            bm = np.asarray(r["bmin"], dtype=np.float32)    # [128, 8192] (-d^2)
            a_d2 = -aw.max(axis=2)                          # [128, 32]
            # row index = i*128 + p  ->  [32, 128] -> flat
            a_rows.append(a_d2.T.reshape(-1))
            cb = -bm.max(axis=0)                            # [8192]
            b_sq = cb if b_sq is None else np.minimum(b_sq, cb)
        a_sq = np.concatenate(a_rows)
        da = np.sqrt(np.maximum(a_sq, 0.0))
        db = np.sqrt(np.maximum(b_sq, 0.0))
        cham.append(da.mean() + db.mean())

    return np.float32(np.mean(cham) / 12.8)
